# revision 1
# baseline (speedup 1.0000x reference)
"""Trainium2 Bass kernel for GAT-style exercise->KC message passing.

Math (per reference):
  kc_Wh = kc_h @ W1                      [1024, 256]
  ex_score[i] = (exercise_h @ W1 @ a[:256])[i]   (scalar per exercise row)
  kc_score[j] = (kc_Wh @ a[256:])[j]
  e[i,j]   = leaky_relu(ex_score[i] + kc_score[j], 0.2)
  p[i,j]   = exp(e[i,j]) * adj[i,j]          (0/1 mask after exp == -inf mask)
  attn     = p / rowsum(p)
  out      = elu((attn @ kc_Wh) * (exercise_h @ E))

Sharding: exercise rows split 8 ways; kc_h/W1/E/a replicated.
Device layout: scores with KC on partitions (the attention matmul then needs
no transposes: lhsT = p block, EX lands on output partitions). The softmax
denominator rides the attention matmul as an appended ones column.
ELU(z) = max(z, exp(min(z, 0)) - 1).
Raw bass (explicit semaphores); host work is shard/pad/transpose/pack only.
"""

import sys

sys.path.insert(0, "/opt/trn_rl_repo")

import numpy as np

N_CORES = 8
N_EX = 50000
N_KC = 1024
D = 256
SHARD = N_EX // N_CORES          # 6250
PAD = 6272                       # 49 * 128
BLOCKS = PAD // 128              # 49
HALVES = [(0, 3200), (3200, 3072)]   # (col offset, width); blocks 0..24 / 25..48
ALPHA = 0.2
WPK = 1808                       # packed consts width

_CACHE = {}


def _build_nc(sim_safe=False, dbg=()):
    import concourse.bass as bass
    import concourse.mybir as mybir

    f32 = mybir.dt.float32
    bf16 = mybir.dt.bfloat16
    i32 = mybir.dt.int32
    AF = mybir.ActivationFunctionType
    ALU = mybir.AluOpType
    X = mybir.AxisListType.X

    nc = bass.Bass()

    exT_d = nc.declare_dram_parameter("exT", [D, PAD], f32, isOutput=False)
    adjT_d = nc.declare_dram_parameter("adjT", [N_KC, PAD], i32, isOutput=False)
    wpack_d = nc.declare_dram_parameter("wpack", [D, WPK], f32, isOutput=False)
    e_d = nc.declare_dram_parameter("eMat", [D, D], f32, isOutput=False)
    out_d = nc.declare_dram_parameter("out", [PAD, D], f32, isOutput=True)
    exrow_s = nc.dram_tensor("exrow_s", [1, PAD], bf16)   # bounce for broadcast

    NG = (BLOCKS + 1) // 2        # 25 groups of <=2 blocks
    adj_tiles = [(h, j) for h in range(2) for j in range(8)]

    from contextlib import ExitStack

    es = ExitStack()
    _ctr = [0]

    def _nm(pfx):
        _ctr[0] += 1
        return f"{pfx}{_ctr[0]}"

    sb = lambda shape, dt: es.enter_context(nc.sbuf_tensor(_nm("t"), shape, dt))
    ps = lambda shape, dt: es.enter_context(nc.psum_tensor(_nm("p"), shape, dt))
    sem = lambda: es.enter_context(nc.semaphore(name=_nm("s")))

    with es:
        wp0 = sb([128, WPK], f32); wp1 = sb([128, WPK], f32)
        exT0 = sb([128, PAD], bf16); exT1 = sb([128, PAD], bf16)
        ebf0 = sb([128, D], bf16); ebf1 = sb([128, D], bf16)
        a2b = sb([128, D], f32)
        w1a1c = sb([128, 2], bf16)
        kcwhE_all = sb([128, 8 * 264], bf16)
        kc_score = sb([128, 8], f32)
        kcs_tmp = sb([128, D], f32)
        ex_row = sb([1, PAD], bf16)
        exb = sb([128, 3200], bf16)
        Lt = sb([128, 3200], f32)
        pm_all = sb([128, 8 * PAD], bf16)
        adjt_all = sb([128, 2 * 3200], bf16)
        recip2 = sb([128, 4], f32)
        ehs2 = sb([128, 3 * D], f32)
        zb2 = sb([128, 2 * 512], f32)
        mb2 = sb([128, 2 * 512], f32)
        e2b = sb([128, 512], f32)
        ps_kcwh = ps([128, D], f32)
        ps_scratch = ps([128, 512], f32)
        ps_att = ps([128, 3 * 512], f32)
        ps_eh = ps([128, 3 * 512], f32)
        (s_d_wp, s_d_misc, s_w1a1t, s_w1a1c, s_kcwh, s_kcj, s_exsc, s_exrow,
         s_bounce, s_exb, s_lrelu, s_exp, s_adj, s_pm, s_blk, s_zdone,
         s_min, s_e2, s_ob, s_store, s_vd, s_adj1) = [sem() for _ in range(22)]
        block = es.enter_context(nc.Block())
        wp = [wp0, wp1]
        exT = [exT0, exT1]
        ebf = [ebf0, ebf1]
        kcwhE = [kcwhE_all[:, 264 * j : 264 * j + 258] for j in range(8)]
        pm = [pm_all[:, PAD * j : PAD * (j + 1)] for j in range(8)]
        adjt = [adjt_all[:, 3200 * k : 3200 * (k + 1)] for k in range(2)]
        ehs = [ehs2[:, D * k : D * (k + 1)] for k in range(3)]
        zb = [zb2[:, 512 * k : 512 * (k + 1)] for k in range(2)]
        mb = [mb2[:, 512 * k : 512 * (k + 1)] for k in range(2)]
        att = [ps_att[:, 512 * k : 512 * k + 258] for k in range(3)]
        eh = [ps_eh[:, 512 * k : 512 * k + D] for k in range(3)]
        ps_w1a1 = ps_scratch[:, 0:2]
        ps_exsc = ps_scratch[0:1, 0:512]
        w1 = [wp[t][:, 0:D] for t in range(2)]
        kchT = [wp[t][:, 2 * D : 2 * D + N_KC] for t in range(2)]
        a1col = [wp[t][:, 1536:1537] for t in range(2)]

        ex_chunks = []   # (lo, w) 512-chunks for ex_score
        lo = 0
        while lo < PAD:
            w = min(512, PAD - lo)
            ex_chunks.append((lo, w))
            lo += w
        NCH = len(ex_chunks)

        def half_of(b):
            return 0 if b < 25 else 1

        # ---------------- SYNC: HWDGE DMAs ----------------
        @block.sync
        def _(sync):
            sync.dma_start(out=wp0[:, :], in_=wpack_d[0:128, :]).then_inc(s_d_wp, 16)
            sync.dma_start(out=wp1[:, :], in_=wpack_d[128:256, :]).then_inc(s_d_wp, 16)
            sync.dma_start(
                out=a2b[:, :],
                in_=wpack_d[0:1, 1537 : 1537 + D].to_broadcast((128, D)),
            ).then_inc(s_d_wp, 16)
            # ex_row -> DRAM bounce -> broadcast loads
            sync.wait_ge(s_exrow, NCH)
            sync.dma_start(out=exrow_s[0:1, :], in_=ex_row[0:1, :]).then_inc(
                s_bounce, 16
            )
            sync.wait_ge(s_bounce, 16)
            sync.dma_start(
                out=exb[:, : HALVES[0][1]],
                in_=exrow_s[0:1, 0 : HALVES[0][1]].to_broadcast((128, HALVES[0][1])),
            ).then_inc(s_exb, 16)
            sync.wait_ge(s_lrelu, 8)   # h0 prelus have read exb
            sync.dma_start(
                out=exb[:, : HALVES[1][1]],
                in_=exrow_s[0:1, HALVES[1][0] : PAD].to_broadcast(
                    (128, HALVES[1][1])
                ),
            ).then_inc(s_exb, 16)
            # output stores
            ns = 0
            for g in range(NG):
                sync.wait_ge(s_ob, g + 1)
                for q in range(2):
                    b = 2 * g + q
                    if b >= BLOCKS:
                        continue
                    sw = 16 if "skip_store" in dbg else 256
                    sync.dma_start(
                        out=out_d[128 * b : 128 * b + 1, :sw].rearrange("a b -> a b") if False else out_d[128 * b : 128 * (b + 1), :sw],
                        in_=mb2[
                            :,
                            512 * (g % 2) + 256 * q : 512 * (g % 2) + 256 * q + sw,
                        ],
                    ).then_inc(s_store, 16)
                    ns += 1
            sync.wait_ge(s_store, 16 * ns)

        # ---------------- GPSIMD: SWDGE cast DMAs + mask ----------------
        @block.gpsimd
        def _(gp):
            gp.dma_start(out=ebf0[:, :], in_=e_d[0:128, :]).then_inc(s_d_misc, 16)
            gp.dma_start(out=ebf1[:, :], in_=e_d[128:256, :]).then_inc(s_d_misc, 16)
            xw = 64 if "skip_exTdma" in dbg else PAD
            gp.dma_start(out=exT0[:, :xw], in_=exT_d[0:128, :xw]).then_inc(s_d_misc, 16)
            gp.dma_start(out=exT1[:, :xw], in_=exT_d[128:256, :xw]).then_inc(s_d_misc, 16)

            def issue_adj(idx):
                h, j = adj_tiles[idx]
                hlo, hw = HALVES[h]
                dw = 64 if "skip_adjdma" in dbg else hw
                gp.dma_start(
                    out=adjt[idx % 2][:, :dw],
                    in_=adjT_d[128 * j : 128 * (j + 1), hlo : hlo + dw],
                ).then_inc(s_adj if idx % 2 == 0 else s_adj1, 16)

            issue_adj(0)
            issue_adj(1)
            for idx in range(16):
                h, j = adj_tiles[idx]
                hlo, hw = HALVES[h]
                gp.wait_ge(s_adj if idx % 2 == 0 else s_adj1, 16 * (idx // 2 + 1))
                gp.wait_ge(s_exp, idx + 1)
                if "skip_mask" in dbg:
                    gp.tensor_tensor(
                        out=pm[j][:, hlo : hlo + 64],
                        in0=pm[j][:, hlo : hlo + 64],
                        in1=adjt[idx % 2][:, :64],
                        op=ALU.mult,
                    ).then_inc(s_pm, 1)
                else:
                    gp.tensor_tensor(
                        out=pm[j][:, hlo : hlo + hw],
                        in0=pm[j][:, hlo : hlo + hw],
                        in1=adjt[idx % 2][:, :hw],
                        op=ALU.mult,
                    ).then_inc(s_pm, 1)
                if idx + 2 < 16:
                    gp.wait_ge(s_pm, idx + 1)
                    issue_adj(idx + 2)

        # ---------------- PE: all matmuls ----------------
        @block.tensor
        def _(pe):
            pe.wait_ge(s_d_wp, 48)
            # w1a1 column [128, 2]: col t = W1T[:, tslice].T @ a1 = (W1 @ a1)[tslice]
            for t in range(2):
                for kt in range(2):
                    mm = nc.tensor.matmul(
                        ps_w1a1[:, t : t + 1] if False else ps_scratch[:, t : t + 1],
                        wp[kt][:, D + 128 * t : D + 128 * (t + 1)],
                        a1col[kt],
                        start=(kt == 0),
                        stop=(kt == 1),
                    )
                    if t == 1 and kt == 1:
                        mm.then_inc(s_w1a1t, 1)
            # kc_Wh per j (single psum buffer; DVE drains each)
            for j in range(8):
                if j >= 1:
                    pe.wait_ge(s_kcj, j)
                for t in range(2):
                    mm = nc.tensor.matmul(
                        ps_kcwh[:, :],
                        kchT[t][:, 128 * j : 128 * (j + 1)],
                        w1[t],
                        start=(t == 0),
                        stop=(t == 1),
                    )
                    if t == 1:
                        mm.then_inc(s_kcwh, 1)
            # ex_score chunks
            pe.wait_ge(s_w1a1c, 1)
            pe.wait_ge(s_d_misc, 64)
            for s, (lo, w) in enumerate(ex_chunks):
                if s >= 1:
                    pe.wait_ge(s_exrow, s)
                for t in range(2):
                    mm = nc.tensor.matmul(
                        ps_scratch[0:1, :w],
                        w1a1c[:, t : t + 1],
                        exT[t][:, lo : lo + w],
                        start=(t == 0),
                        stop=(t == 1),
                    )
                    if t == 1:
                        mm.then_inc(s_exsc, 1)
            # main: attention + Eh per block
            pe.wait_ge(s_kcj, 8)
            for b in range(BLOCKS):
                k = b % 3
                pe.wait_ge(s_pm, 8 if half_of(b) == 0 else 16)
                if b >= 3:
                    pe.wait_ge(s_zdone, b - 2)
                aw2 = 16 if "skip_attmm" in dbg else 258
                for j in range(8):
                    nc.tensor.matmul(
                        att[k][:, 0:aw2],
                        pm[j][:, 128 * b : 128 * (b + 1)],
                        kcwhE[j][:, 0:aw2],
                        start=(j == 0),
                        stop=(j == 7),
                    )
                ew2 = 16 if "skip_ehmm" in dbg else D
                for t in range(2):
                    mm = nc.tensor.matmul(
                        eh[k][:, 0:ew2],
                        exT[t][:, 128 * b : 128 * (b + 1)],
                        ebf[t][:, 0:ew2],
                        start=(t == 0),
                        stop=(t == 1),
                    )
                    if t == 1:
                        mm.then_inc(s_blk, 1)

        # ---------------- DVE ----------------
        @block.vector
        def _(dv):
            vd_n = [0]
            dv.wait_ge(s_d_wp, 48)  # a2b present
            # kc_Wh drain: copy->bf16, ones col, kc_score
            for j in range(8):
                dv.wait_ge(s_kcwh, j + 1)
                nc.vector.tensor_copy(out=kcwhE[j][:, 0:D], in_=ps_kcwh[:, :])
                nc.vector.memset(kcwhE[j][:, D : D + 1], 1.0)
                nc.vector.memset(kcwhE[j][:, D + 1 : D + 2], 0.0)
                nc.vector.tensor_tensor(
                    out=kcs_tmp[:, :], in0=ps_kcwh[:, :], in1=a2b[:, :], op=ALU.mult
                ).then_inc(s_vd, 1)
                vd_n[0] += 1
                dv.wait_ge(s_vd, vd_n[0])
                nc.vector.reduce_sum(
                    kc_score[:, j : j + 1], kcs_tmp[:, :], axis=X
                ).then_inc(s_kcj, 1)
            # w1a1c
            dv.wait_ge(s_w1a1t, 1)
            nc.vector.tensor_copy(out=w1a1c[:, :], in_=ps_scratch[:, 0:2]).then_inc(
                s_w1a1c, 1
            )
            # ex_row chunks
            for s, (lo, w) in enumerate(ex_chunks):
                dv.wait_ge(s_exsc, s + 1)
                nc.vector.tensor_copy(
                    out=ex_row[0:1, lo : lo + w], in_=ps_scratch[0:1, :w]
                ).then_inc(s_exrow, 1)

            # main epilogue
            def out_stt(g2):
                w2 = 512 if 2 * g2 + 1 < BLOCKS else 256
                if "skip_epi" in dbg:
                    w2 = 16
                dv.wait_ge(s_min, g2 + 1)
                dv.wait_ge(s_e2, g2 + 1)
                nc.vector.scalar_tensor_tensor(
                    out=mb[g2 % 2][:, :w2],
                    in0=e2b[:, :w2],
                    scalar=-1.0,
                    in1=zb[g2 % 2][:, :w2],
                    op0=ALU.add,
                    op1=ALU.max,
                ).then_inc(s_ob, 1)

            pending = []
            for b in range(BLOCKS):
                k = b % 3
                g, q = divmod(b, 2)
                dv.wait_ge(s_blk, b + 1)
                ew = 16 if "skip_epi" in dbg else D
                nc.vector.reciprocal(recip2[:, k : k + 1], att[k][:, D : D + 1])
                nc.vector.tensor_copy(out=ehs[k][:, :ew], in_=eh[k][:, :ew]).then_inc(s_vd, 1)
                vd_n[0] += 1
                dv.wait_ge(s_vd, vd_n[0])
                nc.vector.scalar_tensor_tensor(
                    out=zb[g % 2][:, 256 * q : 256 * q + ew],
                    in0=att[k][:, 0:ew],
                    scalar=recip2[:, k : k + 1],
                    in1=ehs[k][:, :ew],
                    op0=ALU.mult,
                    op1=ALU.mult,
                ).then_inc(s_zdone, 1)
                if (q == 1) or (b == BLOCKS - 1):
                    w = 256 * (q + 1)
                    if g >= 2:
                        done_blocks = min(2 * (g - 1), BLOCKS)
                        dv.wait_ge(s_store, 16 * done_blocks)
                    dv.wait_ge(s_zdone, min(2 * g + 2, BLOCKS))
                    if "skip_epi" in dbg:
                        w = 16
                    nc.vector.tensor_scalar_min(
                        mb[g % 2][:, :w], zb[g % 2][:, :w], 0.0
                    ).then_inc(s_min, 1)
                    pending.append(g)
                    if len(pending) >= 2:
                        out_stt(pending.pop(0))
            for g2 in pending:
                out_stt(g2)

        # ---------------- ACT ----------------
        @block.scalar
        def _(act):
            lr_n = [0]
            ex_n = [0]

            def score_item(h, j):
                hlo, hw = HALVES[h]
                act.wait_ge(s_exb, 16 * (h + 1))
                act.wait_ge(s_kcj, j + 1)
                if ex_n[0]:
                    act.wait_ge(s_exp, ex_n[0])   # Lt WAR: prior Exp must retire
                aw = 64 if "skip_act" in dbg else hw
                nc.scalar.activation(
                    Lt[:, :aw],
                    exb[:, :aw],
                    AF.Relu if sim_safe else AF.Prelu,
                    bias=kc_score[:, j : j + 1],
                    scale=1.0,
                    alpha=ALPHA,
                ).then_inc(s_lrelu, 1)
                lr_n[0] += 1
                act.wait_ge(s_lrelu, lr_n[0])
                nc.scalar.activation(
                    pm[j][:, hlo : hlo + aw], Lt[:, :aw], AF.Exp
                ).then_inc(s_exp, 1)
                ex_n[0] += 1

            def elu_item(g):
                w = 512 if 2 * g + 1 < BLOCKS else 256
                act.wait_ge(s_min, g + 1)
                if g >= 1:
                    act.wait_ge(s_ob, g)   # e2b single buffer
                nc.scalar.activation(e2b[:, :w], mb[g % 2][:, :w], AF.Exp).then_inc(
                    s_e2, 1
                )

            for j in range(8):
                score_item(0, j)
            gq = 0
            for j in range(8):
                score_item(1, j)
                if gq < 4:     # interleave a few early groups
                    elu_item(gq)
                    gq += 1
            for g in range(gq, NG):
                elu_item(g)

    return nc


def _prep_shards(exercise_h, kc_h, adj_exercise_kc, W1, E, a):
    exercise_h = np.asarray(exercise_h, dtype=np.float32)
    kc_h = np.asarray(kc_h, dtype=np.float32)
    adj = np.asarray(adj_exercise_kc, dtype=np.int32)
    W1 = np.asarray(W1, dtype=np.float32)
    E = np.asarray(E, dtype=np.float32)
    a = np.asarray(a, dtype=np.float32)

    wpack = np.zeros((D, WPK), dtype=np.float32)
    wpack[:, 0:D] = W1
    wpack[:, D : 2 * D] = W1.T
    wpack[:, 2 * D : 2 * D + N_KC] = kc_h.T
    wpack[:, 1536] = a[:D, 0]
    wpack[0, 1537 : 1537 + D] = a[D:, 0]
    wpack = np.ascontiguousarray(wpack)

    in_maps = []
    for i in range(N_CORES):
        lo = i * SHARD
        exT = np.zeros((D, PAD), dtype=np.float32)
        exT[:, :SHARD] = exercise_h[lo : lo + SHARD].T
        adjT = np.zeros((N_KC, PAD), dtype=np.int32)
        adjT[:, :SHARD] = adj[lo : lo + SHARD].T
        adjT[0, SHARD:] = 1   # keep padded rows' softmax denominator nonzero
        in_maps.append(
            {
                "exT": np.ascontiguousarray(exT),
                "adjT": np.ascontiguousarray(adjT),
                "wpack": wpack,
                "eMat": E,
            }
        )
    return in_maps


def kernel(exercise_h, kc_h, adj_exercise_kc, W1, E, a, _trace=False, _tmpdir=None):
    from concourse.bass_utils import run_bass_kernel_spmd

    if "nc" not in _CACHE:
        _CACHE["nc"] = _build_nc()
    nc = _CACHE["nc"]

    in_maps = _prep_shards(exercise_h, kc_h, adj_exercise_kc, W1, E, a)
    res = run_bass_kernel_spmd(
        nc, in_maps, list(range(N_CORES)), trace=_trace, tmpdir=_tmpdir
    )
    _CACHE["last_result"] = res
    out = np.concatenate(
        [np.asarray(res.results[i]["out"])[:SHARD] for i in range(N_CORES)], axis=0
    )
    return out.astype(np.float32)



# revision 14
# speedup vs baseline: 1.0623x; 1.0623x over previous
"""Trainium2 Bass kernel for GAT-style exercise->KC message passing (v2).

Math (per reference):
  kc_Wh = kc_h @ W1
  z[i,j] = ex_score[i] + kc_score[j]
  p[i,j] = adj * exp(leaky(z)) = adj * max(exp(z), C_i * D_j)
           C = exp(0.2 ex_score), D = exp(0.2 kc_score)
  attn   = p / rowsum(p)   (rowsum via ones column in the attention matmul)
  out    = elu((attn @ kc_Wh) * (exercise_h @ E)), elu(x)=max(x, min(exp(x),1)-1)

Layout: KC on partitions (8 j-chunks of 128), EX on the free axis in 7
stripes of 896 cols. Pipeline per (stripe, j) item:
  ACT exp(z) -> DVE ts (Cb*D_j) + tt max -> mask mult (GPS head / DVE tail)
  -> PE att matmuls per stripe -> DVE/ACT elu epilogue -> SP store.
Eh = exercise_h @ E is computed early by PE and drained to SBUF bf16.
All DRAM I/O bf16/int8; psum fp32. Exercise rows sharded 8 ways.

DMA-completion semaphores can increment out of order across queues, so
every DMA wait is against the FULL count of a dedicated semaphore (or a
parity/slot-split one where issue-order gating bounds the contributors).
"""

import sys

sys.path.insert(0, "/opt/trn_rl_repo")

import numpy as np

N_CORES = 8
N_EX = 50000
N_KC = 1024
D = 256
SHARD = N_EX // N_CORES          # 6250
PAD = 6272                       # 49 * 128
BLOCKS = PAD // 128              # 49
NS = 7                           # stripes
W = PAD // NS                    # 896
BPS = W // 128                   # 7 blocks per stripe
ALPHA = 0.2
WPK = 1808
CHUNK = 448
NCH = PAD // CHUNK               # 14
QW = PAD // 4                    # 1568 exT load quarter
HALVES = [(0, 3584), (3584, 2688)]   # stripe-aligned halves (0-3 / 4-6)
NG = (BLOCKS + 1) // 2           # 25 elu/store groups
MASK_GPS = 512                   # mask cols [0,MASK_GPS) on gpsimd
A_ITEMS = {(s, 4) for s in range(1, 7)}   # items via Prelu+Exp (ACT) path

_CACHE = {}


def _build_nc(dbg=()):
    import concourse.bass as bass
    import concourse.mybir as mybir

    f32 = mybir.dt.float32
    bf16 = mybir.dt.bfloat16
    i8 = mybir.dt.int8
    AF = mybir.ActivationFunctionType
    ALU = mybir.AluOpType

    nc = bass.Bass()

    exT_d = nc.declare_dram_parameter("exT", [D, PAD], bf16, isOutput=False)
    adj_d = nc.declare_dram_parameter("adjT", [N_KC, PAD], i8, isOutput=False)
    wpk_d = nc.declare_dram_parameter("wpack", [D, WPK], bf16, isOutput=False)
    e_d = nc.declare_dram_parameter("eMat", [D, D], bf16, isOutput=False)
    out_d = nc.declare_dram_parameter("out", [PAD, D], bf16, isOutput=True)
    exrow_s = nc.dram_tensor("exrow_s", [1, PAD], bf16)
    crow_s = nc.dram_tensor("crow_s", [1, PAD], bf16)

    from contextlib import ExitStack

    es = ExitStack()
    _ctr = [0]

    def _nm(pfx):
        _ctr[0] += 1
        return f"{pfx}{_ctr[0]}"

    sb = lambda shape, dt: es.enter_context(nc.sbuf_tensor(_nm("t"), shape, dt))
    ps = lambda shape, dt: es.enter_context(nc.psum_tensor(_nm("p"), shape, dt))
    sem = lambda: es.enter_context(nc.semaphore(name=_nm("s")))

    ITEMS = [(s, j) for s in range(NS) for j in range(8)]
    IDX = {it: k for k, it in enumerate(ITEMS)}
    BORD = {}
    nb = 0
    for it in ITEMS:
        if it not in A_ITEMS:
            nb += 1
            BORD[it] = nb
    # per-engine done-counts through item k (inclusive): B on DVE, A on ACT
    NBC, NAC = [], []
    cb_, ca_ = 0, 0
    for it in ITEMS:
        if it in A_ITEMS:
            ca_ += 1
        else:
            cb_ += 1
        NBC.append(cb_)
        NAC.append(ca_)

    ADJH = 3584   # adj ring buffer width; half 1 (2688) reloads at col 0

    def adj_col(s):
        return s * W if s < 4 else s * W - ADJH

    with es:
        # ---- SBUF ----
        wp0 = sb([128, WPK], bf16); wp1 = sb([128, WPK], bf16)
        ebf0 = sb([128, D], bf16); ebf1 = sb([128, D], bf16)
        a2b = sb([128, D], bf16)
        w1a1c = sb([128, 2], bf16)
        kcwhE_all = sb([128, 8 * 264], bf16)
        kc_score = sb([128, 8], f32)
        kcs_tmp = sb([128, D], f32)
        drow = sb([128, 8], f32)
        adj_all = sb([128, 8 * ADJH], bf16)
        exT0 = sb([128, PAD], bf16); exT1 = sb([128, PAD], bf16)
        exb = sb([128, PAD], bf16)
        cb = sb([128, PAD], bf16)
        pm2 = sb([128, 2 * 8 * W], bf16)
        t2b = sb([128, W], bf16)
        ltb = sb([128, W], bf16)
        excp = sb([1, 2 * CHUNK], bf16)
        crow128 = sb([128, 28], bf16)
        crowE = sb([128, 28], bf16)
        ehb = sb([128, BLOCKS * D], bf16)
        dcol = sb([128, 2 * BPS], f32)
        recipb = sb([128, 2 * BPS], f32)
        zb = sb([128, 4 * 512], bf16)
        ebuf = sb([128, 4 * 512], bf16)
        mb = sb([128, 4 * 512], bf16)

        # ---- PSUM (16 KB/partition) ----
        ps_att = ps([128, BPS * 512], f32)

        def eh_slot(b):
            lo = 512 * (3 + b % 4)
            return ps_att[:, lo : lo + D]

        ps_w1a1 = ps_att[:, 1024:1026]
        ps_exsc = [ps_att[0:1, 0:CHUNK], ps_att[0:1, 512 : 512 + CHUNK]]
        ps_kcwh = [ps_att[:, 1536:1792], ps_att[:, 2048:2304]]

        wp = [wp0, wp1]
        exT = [exT0, exT1]
        ebf = [ebf0, ebf1]
        w1 = [wp[t][:, 0:D] for t in range(2)]
        w1T = [wp[t][:, D : 2 * D] for t in range(2)]
        kchT = [wp[t][:, 2 * D : 2 * D + N_KC] for t in range(2)]
        a1col = [wp[t][:, 1536:1537] for t in range(2)]
        kcwhE = [kcwhE_all[:, 264 * j : 264 * j + 258] for j in range(8)]
        adjb = [adj_all[:, ADJH * j : ADJH * (j + 1)] for j in range(8)]

        def pmv(s, j):
            base = (s % 2) * 8 * W + j * W
            return pm2[:, base : base + W]

        s_wp = sem(); s_ebf = sem(); s_a2b = sem()
        s_exq = [sem() for _ in range(4)]
        s_adjt = [sem() for _ in range(8)]
        s_w1a1 = sem(); s_w1a1c = sem(); s_kcwh = sem(); s_kcj = sem()
        s_dj = sem(); s_exsc = sem(); s_excp = sem()
        s_bnc = [sem(), sem()]           # bounce stores by chunk parity
        s_exb = [sem(), sem()]
        s_crl = [sem(), sem()]
        s_cre = sem()
        s_crs = [sem(), sem()]
        s_cb = [sem(), sem()]
        s_pmB = sem(); s_pmA = sem(); s_t2 = sem()
        s_maskG = sem(); s_maskD = sem()
        s_attmm = sem(); s_ehmm = sem(); s_ehcp = sem()
        s_stt = sem(); s_eluE = sem(); s_mb = sem()
        s_st = [sem() for _ in range(4)]   # out stores by group%4
        s_ts = sem(); s_fin = sem(); s_rc = sem(); s_lt = sem()

        block = es.enter_context(nc.Block())

        # ---------------- SYNC (SP): HWDGE plain DMAs ----------------
        @block.sync
        def _(sync):
            sync.dma_start(out=wp0[:, :], in_=wpk_d[0:128, :]).then_inc(s_wp, 16)
            sync.dma_start(out=wp1[:, :], in_=wpk_d[128:256, :]).then_inc(s_wp, 16)
            sync.dma_start(out=ebf0[:, :], in_=e_d[0:128, :]).then_inc(s_ebf, 16)
            sync.dma_start(out=ebf1[:, :], in_=e_d[128:256, :]).then_inc(s_ebf, 16)
            sync.dma_start(
                out=a2b[:, :],
                in_=wpk_d[0:1, 1537 : 1537 + D].to_broadcast((128, D)),
            ).then_inc(s_a2b, 16)
            for q in range(4):
                for t in range(2):
                    sync.dma_start(
                        out=exT[t][:, q * QW : (q + 1) * QW],
                        in_=exT_d[128 * t : 128 * (t + 1), q * QW : (q + 1) * QW],
                    ).then_inc(s_exq[q], 16)
            for c in range(NCH):
                sync.wait_ge(s_excp, c + 1)
                sync.dma_start(
                    out=exrow_s[0:1, c * CHUNK : (c + 1) * CHUNK],
                    in_=excp[0:1, (c % 2) * CHUNK : (c % 2) * CHUNK + CHUNK],
                ).then_inc(s_bnc[c % 2], 16)
            for h in range(2):
                off, hw = HALVES[h]
                kw = hw // 128
                nche = (off + hw) // CHUNK   # 8 / 14 chunks total
                sync.wait_ge(s_bnc[0], 16 * ((nche + 1) // 2))
                sync.wait_ge(s_bnc[1], 16 * (nche // 2))
                sync.dma_start(
                    out=exb[:, off : off + hw],
                    in_=exrow_s[0:1, off : off + hw].to_broadcast((128, hw)),
                ).then_inc(s_exb[h], 16)
                if h == 1:
                    sync.wait_ge(s_cre, 1)   # crow128 reuse (h0 exp done)
                sync.dma_start(
                    out=crow128[:, 0:kw], in_=exrow_s[0:1, off : off + hw]
                ).then_inc(s_crl[h], 16)
                sync.wait_ge(s_cre, h + 1)
                sync.dma_start(
                    out=crow_s[0:1, off : off + hw], in_=crowE[:, 0:kw]
                ).then_inc(s_crs[h], 16)
                sync.wait_ge(s_crs[h], 16)
                sync.dma_start(
                    out=cb[:, off : off + hw],
                    in_=crow_s[0:1, off : off + hw].to_broadcast((128, hw)),
                ).then_inc(s_cb[h], 16)
            stq = [0, 0, 0, 0]
            for g in range(NG):
                gw = 512 if 2 * g + 1 < BLOCKS else 256
                sync.wait_ge(s_mb, g + 1)
                if gw == 512:
                    sync.dma_start(
                        out=out_d[256 * g : 256 * g + 256, :].rearrange(
                            "(q p) c -> p q c", p=128
                        ),
                        in_=mb[:, (g % 4) * 512 : (g % 4) * 512 + 512],
                    ).then_inc(s_st[g % 4], 16)
                else:
                    sync.dma_start(
                        out=out_d[256 * g : 256 * g + 128, :],
                        in_=mb[:, (g % 4) * 512 : (g % 4) * 512 + 256],
                    ).then_inc(s_st[g % 4], 16)
                stq[g % 4] += 1
            for q in range(4):
                sync.wait_ge(s_st[q], 16 * stq[q])

        # ---------------- GPSIMD: SWDGE cast DMAs + mask head ----------------
        @block.gpsimd
        def _(gp):
            for j in range(8):
                gp.dma_start(
                    out=adjb[j][:, 0:ADJH],
                    in_=adj_d[128 * j : 128 * (j + 1), 0:ADJH],
                ).then_inc(s_adjt[j], 16)
            c = MASK_GPS
            for k, (s, j) in enumerate(ITEMS):
                gp.wait_ge(s_pmB, NBC[k])
                if NAC[k]:
                    gp.wait_ge(s_pmA, NAC[k])
                gp.wait_ge(s_adjt[j], 16 if s < 4 else 32)
                lo = adj_col(s)
                gp.tensor_tensor(
                    out=pmv(s, j)[:, 0:c],
                    in0=pmv(s, j)[:, 0:c],
                    in1=adjb[j][:, lo : lo + c],
                    op=ALU.mult,
                ).then_inc(s_maskG, 1)
                if s == 3:
                    gp.wait_ge(s_maskD, IDX[(3, j)] + 1)
                    gp.wait_ge(s_maskG, IDX[(3, j)] + 1)   # own head retired
                    gp.dma_start(
                        out=adjb[j][:, 0 : HALVES[1][1]],
                        in_=adj_d[
                            128 * j : 128 * (j + 1),
                            HALVES[1][0] : HALVES[1][0] + HALVES[1][1],
                        ],
                    ).then_inc(s_adjt[j], 16)

        # ---------------- PE: all matmuls ----------------
        @block.tensor
        def _(pe):
            pe.wait_ge(s_wp, 32)
            for t in range(2):
                for kt in range(2):
                    mm = nc.tensor.matmul(
                        ps_w1a1[:, t : t + 1],
                        w1T[kt][:, 128 * t : 128 * (t + 1)],
                        a1col[kt],
                        start=(kt == 0),
                        stop=(kt == 1),
                    )
                    if t == 1 and kt == 1:
                        mm.then_inc(s_w1a1, 1)
            for j in range(8):
                if j >= 2:
                    pe.wait_ge(s_kcj, j - 1)
                for t in range(2):
                    mm = nc.tensor.matmul(
                        ps_kcwh[j % 2],
                        kchT[t][:, 128 * j : 128 * (j + 1)],
                        w1[t],
                        start=(t == 0),
                        stop=(t == 1),
                    )
                    if t == 1:
                        mm.then_inc(s_kcwh, 1)
            pe.wait_ge(s_w1a1c, 1)
            qdone = -1
            for cix in range(NCH):
                qe = ((cix + 1) * CHUNK - 1) // QW
                while qdone < qe:
                    qdone += 1
                    pe.wait_ge(s_exq[qdone], 32)
                if cix >= 2:
                    pe.wait_ge(s_excp, cix - 1)
                for t in range(2):
                    mm = nc.tensor.matmul(
                        ps_exsc[cix % 2],
                        w1a1c[:, t : t + 1],
                        exT[t][:, cix * CHUNK : (cix + 1) * CHUNK],
                        start=(t == 0),
                        stop=(t == 1),
                    )
                    if t == 1:
                        mm.then_inc(s_exsc, 1)
            # Eh for all blocks (early), rotating through ps_att banks 3-6
            pe.wait_ge(s_ebf, 32)
            pe.wait_ge(s_kcj, 8)   # kcwh drains done (banks 3-4 reuse)
            for b in range(BLOCKS):
                qe = ((b + 1) * 128 - 1) // QW
                while qdone < qe:
                    qdone += 1
                    pe.wait_ge(s_exq[qdone], 32)
                if b >= 4:
                    pe.wait_ge(s_ehcp, b - 3)
                for t in range(2):
                    mm = nc.tensor.matmul(
                        eh_slot(b),
                        exT[t][:, 128 * b : 128 * (b + 1)],
                        ebf[t][:, 0:D],
                        start=(t == 0),
                        stop=(t == 1),
                    )
                    if t == 1:
                        mm.then_inc(s_ehmm, 1)
            # main attention loop
            pe.wait_ge(s_excp, NCH)
            pe.wait_ge(s_ehcp, BLOCKS)   # eh drains done (banks 3-6 reuse)
            for s in range(NS):
                pe.wait_ge(s_maskG, 8 * (s + 1))
                pe.wait_ge(s_maskD, 8 * (s + 1))
                for i in range(BPS):
                    b = s * BPS + i
                    slot = ps_att[:, 512 * i : 512 * i + 258]
                    if b >= BPS:
                        pe.wait_ge(s_stt, b - BPS + 1)
                    for j in range(8):
                        mm = nc.tensor.matmul(
                            slot,
                            pmv(s, j)[:, 128 * i : 128 * (i + 1)],
                            kcwhE[j],
                            start=(j == 0),
                            stop=(j == 7),
                        )
                        if j == 7:
                            mm.then_inc(s_attmm, 1)

        # ---------------- ACT ----------------
        @block.scalar
        def _(act):
            act.wait_ge(s_kcj, 8)
            nc.scalar.activation(
                drow[:, 0:8], kc_score[:, 0:8], AF.Exp, scale=ALPHA
            ).then_inc(s_dj, 1)
            for h in range(2):
                kw = HALVES[h][1] // 128
                act.wait_ge(s_crl[h], 16)
                nc.scalar.activation(
                    crowE[:, 0:kw], crow128[:, 0:kw], AF.Exp, scale=ALPHA
                ).then_inc(s_cre, 1)

            def elu_exp(g):
                gw = 512 if 2 * g + 1 < BLOCKS else 256
                act.wait_ge(s_stt, min(2 * g + 2, BLOCKS))
                if g >= 4:
                    act.wait_ge(s_mb, g - 3)
                nc.scalar.activation(
                    ebuf[:, (g % 4) * 512 : (g % 4) * 512 + gw],
                    zb[:, (g % 4) * 512 : (g % 4) * 512 + gw],
                    AF.Exp,
                ).then_inc(s_eluE, 1)

            g_done = 0
            n_a = 0
            for s in range(NS):
                h = 0 if s < 4 else 1
                for j in range(8):
                    k = IDX[(s, j)]
                    act.wait_ge(s_exb[h], 16)
                    act.wait_ge(s_kcj, j + 1)
                    if s >= 2:
                        act.wait_ge(s_attmm, BPS * (s - 1))
                    src = exb[:, s * W : (s + 1) * W]
                    if (s, j) in A_ITEMS:
                        if n_a >= 1:
                            act.wait_ge(s_pmA, n_a)   # ltb WAR vs prior A Exp
                        nc.scalar.activation(
                            ltb[:, :], src, AF.Prelu,
                            bias=kc_score[:, j : j + 1], scale=1.0, alpha=ALPHA,
                        ).then_inc(s_lt, 1)
                        n_a += 1
                        act.wait_ge(s_lt, n_a)
                        nc.scalar.activation(
                            pmv(s, j)[:, :], ltb[:, :], AF.Exp
                        ).then_inc(s_pmA, 1)
                    else:
                        nc.scalar.activation(
                            pmv(s, j)[:, :], src, AF.Exp,
                            bias=kc_score[:, j : j + 1], scale=1.0,
                        ).then_inc(s_t2, 1)
                if s >= 1:
                    g_hi = (BPS * s - 2) // 2
                    while g_done <= g_hi:
                        elu_exp(g_done)
                        g_done += 1
            while g_done < NG:
                elu_exp(g_done)
                g_done += 1

        # ---------------- DVE ----------------
        @block.vector
        def _(dv):
            dv.wait_ge(s_w1a1, 1)
            nc.vector.tensor_copy(out=w1a1c[:, :], in_=ps_w1a1).then_inc(s_w1a1c, 1)
            dv.wait_ge(s_a2b, 16)
            for j in range(8):
                dv.wait_ge(s_kcwh, j + 1)
                if j >= 1:
                    dv.wait_ge(s_kcj, j)   # serialize kcs_tmp WAW
                nc.vector.tensor_copy(out=kcwhE[j][:, 0:D], in_=ps_kcwh[j % 2])
                nc.vector.memset(kcwhE[j][:, D : D + 1], 1.0)
                nc.vector.memset(kcwhE[j][:, D + 1 : D + 2], 0.0)
                nc.vector.scalar_tensor_tensor(
                    out=kcs_tmp[:, :],
                    in0=ps_kcwh[j % 2],
                    scalar=1.0,
                    in1=a2b[:, :],
                    op0=ALU.mult,
                    op1=ALU.mult,
                    accum_out=kc_score[:, j : j + 1],
                ).then_inc(s_kcj, 1)
            for c in range(NCH):
                dv.wait_ge(s_exsc, c + 1)
                if c >= 2:
                    dv.wait_ge(s_bnc[c % 2], 16 * (c // 2))
                nc.vector.tensor_copy(
                    out=excp[0:1, (c % 2) * CHUNK : (c % 2) * CHUNK + CHUNK],
                    in_=ps_exsc[c % 2],
                ).then_inc(s_excp, 1)

            eh_next = [0]

            def drain_eh(n):
                while eh_next[0] < min(n, BLOCKS):
                    b = eh_next[0]
                    dv.wait_ge(s_ehmm, b + 1)
                    nc.vector.tensor_copy(
                        out=ehb[:, D * b : D * (b + 1)],
                        in_=eh_slot(b),
                    ).then_inc(s_ehcp, 1)
                    eh_next[0] += 1

            dv.wait_ge(s_dj, 1)
            pend_fin = []
            n_ts = [0]    # s_ts incs
            n_fin = [0]   # s_fin incs
            n_rc = [0]    # s_rc incs (gathers)
            deferred = [None]   # deferred mask-tail closure

            def flush_tail():
                if deferred[0] is not None:
                    fn = deferred[0]
                    deferred[0] = None
                    fn()

            def drain_fins():
                # software-pipelined pairs: all F (ts) first, then all maxes
                gs = list(pend_fin)
                pend_fin.clear()
                for g in gs:
                    gw = 512 if 2 * g + 1 < BLOCKS else 256
                    dv.wait_ge(s_eluE, g + 1)
                    nc.vector.tensor_scalar(
                        out=ebuf[:, (g % 4) * 512 : (g % 4) * 512 + gw],
                        in0=ebuf[:, (g % 4) * 512 : (g % 4) * 512 + gw],
                        scalar1=1.0,
                        scalar2=-1.0,
                        op0=ALU.min,
                        op1=ALU.add,
                    ).then_inc(s_fin, 1)
                    n_fin[0] += 1
                for g in gs:
                    gw = 512 if 2 * g + 1 < BLOCKS else 256
                    dv.wait_ge(s_fin, n_fin[0] - len(gs) + gs.index(g) + 1)
                    nc.vector.tensor_tensor(
                        out=mb[:, (g % 4) * 512 : (g % 4) * 512 + gw],
                        in0=zb[:, (g % 4) * 512 : (g % 4) * 512 + gw],
                        in1=ebuf[:, (g % 4) * 512 : (g % 4) * 512 + gw],
                        op=ALU.max,
                    ).then_inc(s_mb, 1)

            def do_stt(b):
                s2, i2 = divmod(b, BPS)
                g, qq = divmod(b, 2)
                if g >= 4:
                    dv.wait_ge(s_st[g % 4], 16 * (g // 4))   # zb slot reuse
                dv.wait_ge(s_ehcp, b + 1)    # ehb[b] drained (same-engine edge)
                dv.wait_ge(s_rc, s2 + 1)     # recip of stripe s2 retired
                nc.vector.scalar_tensor_tensor(
                    out=zb[:, (g % 4) * 512 + 256 * qq : (g % 4) * 512 + 256 * qq + 256],
                    in0=ps_att[:, 512 * i2 : 512 * i2 + 256],
                    scalar=recipb[:, (s2 % 2) * BPS + i2 : (s2 % 2) * BPS + i2 + 1],
                    in1=ehb[:, D * b : D * (b + 1)],
                    op0=ALU.mult,
                    op1=ALU.mult,
                ).then_inc(s_stt, 1)
                if qq == 1 or b == BLOCKS - 1:
                    pend_fin.append(g)

            def gather_recip(s2):
                dv.wait_ge(s_attmm, BPS * (s2 + 1))
                if s2 >= 2:
                    dv.wait_ge(s_stt, BPS * (s2 - 1))   # recipb slot WAR
                nc.vector.reciprocal(
                    recipb[:, (s2 % 2) * BPS : (s2 % 2) * BPS + BPS],
                    ps_att[:, 256 : BPS * 512 : 512],
                ).then_inc(s_rc, 1)
                n_rc[0] += 1

            for s in range(NS):
                if s >= 1:
                    drain_fins()
                    gather_recip(s - 1)
                for j in range(8):
                    k = IDX[(s, j)]
                    if s == 0:
                        drain_eh(6 * (j + 1) + 1)
                    if (s, j) in A_ITEMS:
                        flush_tail()
                        dv.wait_ge(s_pmA, NAC[k])
                    else:
                        dv.wait_ge(s_t2, BORD[(s, j)])
                        dv.wait_ge(s_cb[0 if s < 4 else 1], 16)
                        if BORD[(s, j)] >= 2:
                            dv.wait_ge(s_pmB, BORD[(s, j)] - 1)   # t2b WAR
                        nc.vector.tensor_scalar(
                            out=t2b[:, :],
                            in0=cb[:, s * W : (s + 1) * W],
                            scalar1=drow[:, j : j + 1],
                            scalar2=None,
                            op0=ALU.mult,
                        ).then_inc(s_ts, 1)
                        n_ts[0] += 1
                        flush_tail()   # fill the ts->max latency
                        dv.wait_ge(s_ts, n_ts[0])
                        nc.vector.tensor_tensor(
                            out=pmv(s, j)[:, :],
                            in0=pmv(s, j)[:, :],
                            in1=t2b[:, :],
                            op=ALU.max,
                        ).then_inc(s_pmB, 1)

                    def mk_tail(s=s, j=j, k=k):
                        def fn():
                            if (s, j) in A_ITEMS:
                                dv.wait_ge(s_pmA, NAC[k])
                            else:
                                dv.wait_ge(s_pmB, NBC[k])
                            dv.wait_ge(s_adjt[j], 16 if s < 4 else 32)
                            lo = adj_col(s)
                            nc.vector.tensor_tensor(
                                out=pmv(s, j)[:, MASK_GPS:W],
                                in0=pmv(s, j)[:, MASK_GPS:W],
                                in1=adjb[j][:, lo + MASK_GPS : lo + W],
                                op=ALU.mult,
                            ).then_inc(s_maskD, 1)
                        return fn

                    deferred[0] = mk_tail()
                    if s >= 1 and j < BPS:
                        do_stt(BPS * (s - 1) + j)
                flush_tail()
                drain_eh(BLOCKS)
            drain_fins()
            gather_recip(NS - 1)
            for b in range(BPS * (NS - 1), BLOCKS):
                do_stt(b)
                drain_fins()

    return nc


def _prep_shards(exercise_h, kc_h, adj_exercise_kc, W1, E, a):
    import ml_dtypes

    bf16 = ml_dtypes.bfloat16
    exercise_h = np.asarray(exercise_h, dtype=np.float32)
    kc_h = np.asarray(kc_h, dtype=np.float32)
    adj = np.asarray(adj_exercise_kc, dtype=np.int8)
    W1 = np.asarray(W1, dtype=np.float32)
    E = np.asarray(E, dtype=np.float32)
    a = np.asarray(a, dtype=np.float32)

    wpack = np.zeros((D, WPK), dtype=np.float32)
    wpack[:, 0:D] = W1
    wpack[:, D : 2 * D] = W1.T
    wpack[:, 2 * D : 2 * D + N_KC] = kc_h.T
    wpack[:, 1536] = a[:D, 0]
    wpack[0, 1537 : 1537 + D] = a[D:, 0]
    wpack = np.ascontiguousarray(wpack.astype(bf16))
    eM = np.ascontiguousarray(E.astype(bf16))

    in_maps = []
    for i in range(N_CORES):
        lo = i * SHARD
        exT = np.zeros((D, PAD), dtype=bf16)
        exT[:, :SHARD] = exercise_h[lo : lo + SHARD].T.astype(bf16)
        adjT = np.zeros((N_KC, PAD), dtype=np.int8)
        adjT[:, :SHARD] = adj[lo : lo + SHARD].T
        adjT[0, SHARD:] = 1
        in_maps.append(
            {
                "exT": np.ascontiguousarray(exT),
                "adjT": np.ascontiguousarray(adjT),
                "wpack": wpack,
                "eMat": eM,
            }
        )
    return in_maps


def kernel(exercise_h, kc_h, adj_exercise_kc, W1, E, a, _trace=False, _tmpdir=None):
    from concourse.bass_utils import run_bass_kernel_spmd

    if "nc" not in _CACHE:
        _CACHE["nc"] = _build_nc()
    nc = _CACHE["nc"]

    in_maps = _prep_shards(exercise_h, kc_h, adj_exercise_kc, W1, E, a)
    res = run_bass_kernel_spmd(
        nc, in_maps, list(range(N_CORES)), trace=_trace, tmpdir=_tmpdir
    )
    _CACHE["last_result"] = res
    out = np.concatenate(
        [
            np.asarray(res.results[i]["out"])[:SHARD].astype(np.float32)
            for i in range(N_CORES)
        ],
        axis=0,
    )
    return out
